# revision 11
# baseline (speedup 1.0000x reference)
"""Trainium2 Bass kernel: Coarse_layer (segment_reduce).

Computes, on 8 NeuronCores:
  coarse_edge_attrs     = scatter_mean(edge_attr, fine2coarse_edges, E_COARSE)
  fine_node_process     = x + MLP(concat([x, distances]))
  coarse_node_attrs_avg = scatter_mean(fine_node_process, fine2coarse_index, N_COARSE)
  (coarse_edge_index passed through)

Sharding: host sorts fine rows by coarse segment id and partitions the coarse
segment space into 128-segment blocks; blocks are dealt to the 8 cores bucketed
by their subtile count so every core gets an IDENTICAL program structure (one
SPMD program, per-core data).  Each block's segment-sum is computed on the PE
as  onehot^T @ vals  accumulated in PSUM, where the one-hot selection matrix is
built on DVE/GPSIMD via tensor_scalar(is_equal) against an iota row.  Segment
boundaries never cross cores, so no collectives are needed.
"""

import math
import os
import sys

import numpy as np

if "/opt/trn_rl_repo" not in sys.path:
    sys.path.insert(0, "/opt/trn_rl_repo")

import ml_dtypes

# ---------------------------------------------------------------- constants
N_FINE = 400000
E_FINE = 1600000
N_COARSE = 100000
E_COARSE = 400000
H = 128
P = 128
NCORES = 8
OUT_BATCH = 4

# knobs
COMPUTE = os.environ.get("KERNEL_DTYPE", "bf16")  # "bf16" | "f32"
TRACE = os.environ.get("KERNEL_TRACE", "0") == "1"
# engines used for one-hot builds, cycled per subtile
OH_ENGINES = os.environ.get("KERNEL_OH_ENGINES", "vector,gpsimd,vector").split(",")


def _npdt():
    return ml_dtypes.bfloat16 if COMPUTE == "bf16" else np.float32


# ---------------------------------------------------------------- host plan
def _plan(seg_ids, n_segments):
    """Partition a scatter into per-core block schedules with a shared profile.

    Returns dict with:
      prof      [NB]              subtile count per block position (same all cores)
      slot      [NCORES, T*P]     source-row id per slot, -1 = padding
      offs      [NCORES, T*P]     local segment offset (0..127) per slot, -1 pad
      invc      [NCORES, NB, P]   1/max(count,1) per segment
      cnts      [NCORES, NB, P]   count per segment (0 for dummy blocks)
      outmap    [NCORES, NB]      real block id or -1
      nblk      total real blocks
    """
    n = seg_ids.shape[0]
    order = np.argsort(seg_ids, kind="stable")
    counts = np.bincount(seg_ids, minlength=n_segments).astype(np.int64)
    nblk = -(-n_segments // P)
    counts_p = np.zeros(nblk * P, np.int64)
    counts_p[:n_segments] = counts
    blk_cnt = counts_p.reshape(nblk, P).sum(1)
    nsub = np.maximum(1, -(-blk_cnt // P)).astype(np.int64)
    seg_start = np.zeros(nblk * P + 1, np.int64)
    np.cumsum(counts_p, out=seg_start[1:])

    per_core = [[] for _ in range(NCORES)]
    for nv in sorted(set(nsub.tolist()), reverse=True):
        bids = np.where(nsub == nv)[0].tolist()
        while len(bids) % NCORES:
            bids.append(-1)
        for i, bid in enumerate(bids):
            per_core[i % NCORES].append((int(bid), int(nv)))

    prof = [nv for (_, nv) in per_core[0]]
    NB = len(prof)
    T = sum(prof)
    slot = np.full((NCORES, T * P), -1, np.int64)
    offs = np.full((NCORES, T * P), -1.0, np.float32)
    invc = np.ones((NCORES, NB, P), np.float32)
    cnts = np.zeros((NCORES, NB, P), np.float32)
    outmap = np.full((NCORES, NB), -1, np.int64)
    for c in range(NCORES):
        t0 = 0
        for j, (bid, nv) in enumerate(per_core[c]):
            assert prof[j] == nv
            if bid >= 0:
                a = int(seg_start[bid * P])
                cnt = int(blk_cnt[bid])
                rows = order[a : a + cnt]
                slot[c, t0 * P : t0 * P + cnt] = rows
                offs[c, t0 * P : t0 * P + cnt] = (
                    seg_ids[rows] - bid * P
                ).astype(np.float32)
                invc[c, j] = 1.0 / np.maximum(counts_p[bid * P : (bid + 1) * P], 1)
                cnts[c, j] = counts_p[bid * P : (bid + 1) * P]
                outmap[c, j] = bid
            t0 += nv
    return dict(
        prof=prof, slot=slot, offs=offs, invc=invc, cnts=cnts, outmap=outmap,
        nblk=nblk, NB=NB, T=T,
    )


def _gather_block_layout(V, slot, dt):
    """V [N, H] -> [P, T*H] where out[e, t*H + f] = V[slot[t*P + e], f] (0 pad)."""
    TP = slot.shape[0]
    T = TP // P
    out = np.zeros((TP, V.shape[1]), dtype=dt)
    m = slot >= 0
    out[m] = V[slot[m]].astype(dt)
    return np.ascontiguousarray(
        out.reshape(T, P, V.shape[1]).transpose(1, 0, 2).reshape(P, T * V.shape[1])
    )


def _offs_layout(offs, dt):
    """[T*P] -> [P, T] with out[p, t] = offs[t*P + p]."""
    T = offs.shape[0] // P
    return np.ascontiguousarray(offs.reshape(T, P).T.astype(dt))


# ---------------------------------------------------------------- device program
def _build_program(prof_e, NB_e, prof_n, NB_n):
    from contextlib import ExitStack

    import concourse.tile as tile
    from concourse import bacc, bass, mybir

    DT = mybir.dt.bfloat16 if COMPUTE == "bf16" else mybir.dt.float32
    F32 = mybir.dt.float32
    T_e = sum(prof_e)
    T_n = sum(prof_n)

    nc = bacc.Bacc(
        "TRN2", target_bir_lowering=False, debug=False, num_devices=NCORES
    )

    def din(name, shape, dt=DT):
        return nc.declare_dram_parameter(name, list(shape), dt, isOutput=False).ap()

    vals_e = din("vals_e", [P, T_e * H])
    offs_e = din("offs_e", [P, T_e])
    invc_e = din("invc_e", [P, NB_e], F32)
    xaug = din("xaug", [P + 2, T_n * P])
    offs_n = din("offs_n", [P, T_n])
    invc_n = din("invc_n", [P, NB_n], F32)
    cnt_n = din("cnt_n", [1, NB_n * P])
    w1a = din("w1a", [P, P])
    w1b = din("w1b", [2, P])
    w2 = din("w2", [P, P])
    b2r = din("b2r", [1, P])
    iota_d = din("iota_d", [P, P])
    ident_d = din("ident_d", [P, P])

    ce_out = nc.declare_dram_parameter(
        "ce_out", [P, NB_e * P], F32, isOutput=True
    ).ap()
    cn_out = nc.declare_dram_parameter(
        "cn_out", [P, NB_n * P], F32, isOutput=True
    ).ap()

    eq = mybir.AluOpType.is_equal
    Relu = mybir.ActivationFunctionType.Relu

    def oh_engine(t):
        name = OH_ENGINES[t % len(OH_ENGINES)]
        return getattr(nc, name)

    with tile.TileContext(nc) as tc, ExitStack() as ctx:
        const = ctx.enter_context(tc.tile_pool(name="const", bufs=1))

        iota_t = const.tile([P, P], DT)
        ident_t = const.tile([P, P], DT)
        w1a_t = const.tile([P, P], DT)
        w1b_t = const.tile([2, P], DT)
        w2_t = const.tile([P, P], DT)
        b2_t = const.tile([1, P], DT)
        offs_e_t = const.tile([P, T_e], DT)
        invc_e_t = const.tile([P, NB_e], F32)
        offs_n_t = const.tile([P, T_n], DT)
        invc_n_t = const.tile([P, NB_n], F32)
        cnt_n_t = const.tile([1, NB_n * P], DT)
        for dst, src in [
            (iota_t, iota_d), (ident_t, ident_d), (w1a_t, w1a), (w1b_t, w1b),
            (w2_t, w2), (b2_t, b2r), (offs_e_t, offs_e), (invc_e_t, invc_e),
            (offs_n_t, offs_n), (invc_n_t, invc_n), (cnt_n_t, cnt_n),
        ]:
            nc.sync.dma_start(out=dst[:], in_=src[:])

        # ---------------- edge phase -------------------------------------
        with tc.tile_pool(name="ev", bufs=3) as vpool, \
             tc.tile_pool(name="eo", bufs=8) as ohpool, \
             tc.tile_pool(name="eout", bufs=3) as outpool, \
             tc.tile_pool(name="eps", bufs=4, space="PSUM") as epsum:
            t = 0
            outt = None
            for b in range(NB_e):
                n = prof_e[b]
                vals = vpool.tile([P, n * H], DT, tag="ev")
                nc.sync.dma_start(out=vals[:], in_=vals_e[:, t * H : (t + n) * H])
                ps = epsum.tile([P, P], F32, tag="eps")
                for s in range(n):
                    oh = ohpool.tile([P, P], DT, tag="eo")
                    oh_engine(t).tensor_tensor(
                        oh[:], iota_t[:],
                        offs_e_t[:, t : t + 1].to_broadcast([P, P]), eq
                    )
                    nc.tensor.matmul(
                        ps[:], lhsT=oh[:], rhs=vals[:, s * H : (s + 1) * H],
                        start=(s == 0), stop=(s == n - 1),
                    )
                    t += 1
                ob = b % OUT_BATCH
                if ob == 0:
                    outt = outpool.tile([P, OUT_BATCH * P], F32, tag="eout")
                nc.scalar.mul(
                    outt[:, ob * P : (ob + 1) * P], ps[:], invc_e_t[:, b : b + 1]
                )
                if ob == OUT_BATCH - 1 or b == NB_e - 1:
                    c0 = (b - ob) * P
                    nc.sync.dma_start(
                        out=ce_out[:, c0 : (b + 1) * P], in_=outt[:, : (ob + 1) * P]
                    )

        # ---------------- node phase -------------------------------------
        with tc.tile_pool(name="nx", bufs=3) as xpool, \
             tc.tile_pool(name="nd", bufs=3) as dpool, \
             tc.tile_pool(name="nh", bufs=4) as hpool, \
             tc.tile_pool(name="nf", bufs=4) as fpool, \
             tc.tile_pool(name="nv", bufs=4) as nvpool, \
             tc.tile_pool(name="no", bufs=8) as nohpool, \
             tc.tile_pool(name="nout", bufs=3) as noutpool, \
             tc.tile_pool(name="mps", bufs=4, space="PSUM") as mpsum, \
             tc.tile_pool(name="tps", bufs=2, space="PSUM") as tpsum, \
             tc.tile_pool(name="cps", bufs=2, space="PSUM") as cpsum:
            t = 0
            outt = None
            for b in range(NB_n):
                n = prof_n[b]
                xs = xpool.tile([P, n * P], DT, tag="nx")
                nc.sync.dma_start(out=xs[:], in_=xaug[0:P, t * P : (t + n) * P])
                da = dpool.tile([2, n * P], DT, tag="nd")
                nc.sync.dma_start(
                    out=da[:], in_=xaug[P : P + 2, t * P : (t + n) * P]
                )
                cps = cpsum.tile([P, P], F32, tag="cps")
                # mean(x + mlp + b2) = (sum(x + mlp) + count*b2) * invc
                # out[seg, feat] += cnt[seg] * b2[feat]
                nc.tensor.matmul(
                    cps[:], lhsT=cnt_n_t[:, b * P : (b + 1) * P], rhs=b2_t[:],
                    start=True, stop=False,
                )
                for s in range(n):
                    sl = slice(s * P, (s + 1) * P)
                    ps1 = mpsum.tile([P, P], F32, tag="mps")
                    nc.tensor.matmul(
                        ps1[:], lhsT=w1a_t[:], rhs=xs[:, sl], start=True, stop=False
                    )
                    nc.tensor.matmul(
                        ps1[:], lhsT=w1b_t[:], rhs=da[:, sl], start=False, stop=True
                    )
                    h = hpool.tile([P, P], DT, tag="nh")
                    nc.scalar.activation(h[:], ps1[:], Relu)
                    ps2 = mpsum.tile([P, P], F32, tag="mps")
                    nc.tensor.matmul(
                        ps2[:], lhsT=w2_t[:], rhs=h[:], start=True, stop=True
                    )
                    ft = fpool.tile([P, P], DT, tag="nf")
                    nc.vector.tensor_tensor(
                        ft[:], ps2[:], xs[:, sl], mybir.AluOpType.add
                    )
                    tp = tpsum.tile([P, P], DT, tag="tps")
                    nc.tensor.transpose(tp[:], ft[:], ident_t[:])
                    vm = nvpool.tile([P, P], DT, tag="nv")
                    if t % 2 == 0:
                        nc.scalar.copy(vm[:], tp[:])
                    else:
                        nc.vector.tensor_copy(vm[:], tp[:])
                    oh = nohpool.tile([P, P], DT, tag="no")
                    oh_engine(t).tensor_tensor(
                        oh[:], iota_t[:],
                        offs_n_t[:, t : t + 1].to_broadcast([P, P]), eq
                    )
                    nc.tensor.matmul(
                        cps[:], lhsT=oh[:], rhs=vm[:],
                        start=False, stop=(s == n - 1),
                    )
                    t += 1
                ob = b % OUT_BATCH
                if ob == 0:
                    outt = noutpool.tile([P, OUT_BATCH * P], F32, tag="nout")
                nc.scalar.mul(
                    outt[:, ob * P : (ob + 1) * P], cps[:], invc_n_t[:, b : b + 1]
                )
                if ob == OUT_BATCH - 1 or b == NB_n - 1:
                    c0 = (b - ob) * P
                    nc.sync.dma_start(
                        out=cn_out[:, c0 : (b + 1) * P], in_=outt[:, : (ob + 1) * P]
                    )
    nc.compile()
    return nc


# ---------------------------------------------------------------- executor
def _execute(nc, in_maps):
    """Run the SPMD program on 8 cores via PJRT (axon).  Mirrors
    bass2jax.run_bass_via_pjrt, but pre-places inputs on device and, when
    TRACE is set, wall-clock-times warm re-executions (no NTFF hook in this
    container)."""
    import time as _time

    import jax
    from jax.experimental.shard_map import shard_map
    from jax.sharding import Mesh, NamedSharding, PartitionSpec

    from concourse import bass2jax, mybir

    bass2jax.install_neuronx_cc_hook()

    part_name = nc.partition_id_tensor.name if nc.partition_id_tensor else None
    in_names, out_names, out_avals, zero_outs = [], [], [], []
    for alloc in nc.m.functions[0].allocations:
        if not isinstance(alloc, mybir.MemoryLocationSet):
            continue
        name = alloc.memorylocations[0].name
        if alloc.kind == "ExternalInput":
            if name != part_name:
                in_names.append(name)
        elif alloc.kind == "ExternalOutput":
            out_names.append(name)
            shape = tuple(alloc.tensor_shape)
            dtype = mybir.dt.np(alloc.dtype)
            out_avals.append(jax.core.ShapedArray(shape, dtype))
            zero_outs.append(np.zeros(shape, dtype))
    n_params = len(in_names)
    all_names = in_names + out_names
    if part_name is not None:
        all_names = all_names + [part_name]

    def _body(*args):
        operands = list(args)
        if part_name is not None:
            operands.append(bass2jax.partition_id_tensor())
        outs = bass2jax._bass_exec_p.bind(
            *operands,
            out_avals=tuple(out_avals),
            in_names=tuple(all_names),
            out_names=tuple(out_names),
            lowering_input_output_aliases=(),
            sim_require_finite=True,
            sim_require_nnan=True,
            nc=nc,
        )
        return tuple(outs)

    devices = jax.devices()[:NCORES]
    mesh = Mesh(np.asarray(devices), ("core",))
    spec = NamedSharding(mesh, PartitionSpec("core"))
    nin = n_params + len(out_names)
    sharded = jax.jit(
        shard_map(
            _body, mesh=mesh, in_specs=(PartitionSpec("core"),) * nin,
            out_specs=(PartitionSpec("core"),) * len(out_names), check_rep=False,
        ),
        keep_unused=True,
    )
    concat_in = [
        jax.device_put(
            np.concatenate([np.asarray(in_maps[c][nm]) for c in range(NCORES)], 0),
            spec,
        )
        for nm in in_names
    ]
    concat_zero = [
        jax.device_put(np.zeros((NCORES * z.shape[0], *z.shape[1:]), z.dtype), spec)
        for z in zero_outs
    ]
    out_arrs = jax.block_until_ready(sharded(*concat_in, *concat_zero))
    if TRACE:
        times = []
        for _ in range(5):
            t0 = _time.perf_counter()
            jax.block_until_ready(sharded(*concat_in, *concat_zero))
            times.append(_time.perf_counter() - t0)
        print(f"HW exec time: {int(min(times) * 1e9)} ns")
        print("exec wall times (ms):", [round(t * 1e3, 3) for t in times])
    return [
        {
            nm: np.asarray(out_arrs[i]).reshape(NCORES, *out_avals[i].shape)[c]
            for i, nm in enumerate(out_names)
        }
        for c in range(NCORES)
    ]


# ---------------------------------------------------------------- runner
def _run(x, coarse_edge_index, edge_attr, fine2coarse_index, fine2coarse_edges,
         distances, W1, b1, W2, b2, n_coarse_nodes, n_coarse_edges):
    dt = _npdt()
    plan_e = _plan(fine2coarse_edges, n_coarse_edges)
    plan_n = _plan(fine2coarse_index, n_coarse_nodes)

    iota = np.broadcast_to(np.arange(P, dtype=np.float32), (P, P)).astype(dt)
    ident = np.eye(P, dtype=np.float32).astype(dt)
    w1a = W1[:P].astype(dt)
    w1b = np.stack([W1[P], b1]).astype(dt)  # [2, P]: distance row + bias row
    w2c = W2.astype(dt)
    b2r = b2[None, :].astype(dt)

    in_maps = []
    for c in range(NCORES):
        # node phase wants feature-major: xs[f, slot] = x[slot_row, f]
        slot_n = plan_n["slot"][c]
        m = slot_n >= 0
        g = np.zeros((slot_n.shape[0], H), np.float32)
        g[m] = x[slot_n[m]]
        xs = np.ascontiguousarray(g.T.astype(dt))  # [H, T_n*P]
        drow = np.zeros(slot_n.shape[0], np.float32)
        drow[m] = distances[slot_n[m], 0]
        ones = m.astype(np.float32)
        daux = np.stack([drow, ones]).astype(dt)  # [2, T_n*P] in slot order
        xaug = np.concatenate([xs, daux], axis=0)
        in_maps.append({
            "vals_e": _gather_block_layout(edge_attr, plan_e["slot"][c], dt),
            "offs_e": _offs_layout(plan_e["offs"][c], dt),
            "invc_e": np.ascontiguousarray(plan_e["invc"][c].T),
            "xaug": np.ascontiguousarray(xaug),
            "offs_n": _offs_layout(plan_n["offs"][c], dt),
            "invc_n": np.ascontiguousarray(plan_n["invc"][c].T),
            "cnt_n": plan_n["cnts"][c].reshape(1, -1).astype(dt),
            "w1a": w1a, "w1b": w1b, "w2": w2c, "b2r": b2r,
            "iota_d": iota, "ident_d": ident,
        })

    nc = _build_program(plan_e["prof"], plan_e["NB"], plan_n["prof"], plan_n["NB"])
    results = _execute(nc, in_maps)

    def assemble(key, plan, n_segments):
        nblk = plan["nblk"]
        NB = plan["NB"]
        full = np.zeros((nblk * P, H), np.float32)
        fullb = full.reshape(nblk, P, H)
        for c in range(NCORES):
            r = results[c][key]  # [P, NB*P]
            r = r.reshape(P, NB, P).transpose(1, 0, 2)  # [NB, seg_in_block, feat]
            msk = plan["outmap"][c] >= 0
            fullb[plan["outmap"][c][msk]] = r[msk]
        return full[:n_segments]

    ce = assemble("ce_out", plan_e, n_coarse_edges)
    cn = assemble("cn_out", plan_n, n_coarse_nodes)
    return cn, ce


def kernel(**inputs):
    x = np.asarray(inputs["x"], np.float32)
    cei = np.asarray(inputs["coarse_edge_index"])
    ea = np.asarray(inputs["edge_attr"], np.float32)
    f2ci = np.asarray(inputs["fine2coarse_index"]).astype(np.int64)
    f2ce = np.asarray(inputs["fine2coarse_edges"]).astype(np.int64)
    dist = np.asarray(inputs["distances"], np.float32)
    W1 = np.asarray(inputs["W1"], np.float32)
    b1 = np.asarray(inputs["b1"], np.float32)
    W2 = np.asarray(inputs["W2"], np.float32)
    b2 = np.asarray(inputs["b2"], np.float32)
    cn, ce = _run(x, cei, ea, f2ci, f2ce, dist, W1, b1, W2, b2,
                  N_COARSE, E_COARSE)
    return cn, ce, cei


# revision 12
# speedup vs baseline: 1.0409x; 1.0409x over previous
"""Trainium2 Bass kernel: Coarse_layer (segment_reduce).

Computes, on 8 NeuronCores:
  coarse_edge_attrs     = scatter_mean(edge_attr, fine2coarse_edges, E_COARSE)
  fine_node_process     = x + MLP(concat([x, distances]))
  coarse_node_attrs_avg = scatter_mean(fine_node_process, fine2coarse_index, N_COARSE)
  (coarse_edge_index passed through)

Sharding: host sorts fine rows by coarse segment id and partitions the coarse
segment space into 128-segment blocks; blocks are dealt to the 8 cores bucketed
by their subtile count so every core gets an IDENTICAL program structure (one
SPMD program, per-core data).  Each block's segment-sum is computed on the PE
as  onehot^T @ vals  accumulated in PSUM, where the one-hot selection matrix is
built on DVE/GPSIMD via tensor_scalar(is_equal) against an iota row.  Segment
boundaries never cross cores, so no collectives are needed.
"""

import math
import os
import sys

import numpy as np

if "/opt/trn_rl_repo" not in sys.path:
    sys.path.insert(0, "/opt/trn_rl_repo")

import ml_dtypes

# ---------------------------------------------------------------- constants
N_FINE = 400000
E_FINE = 1600000
N_COARSE = 100000
E_COARSE = 400000
H = 128
P = 128
NCORES = 8
OUT_BATCH = 4

# knobs
COMPUTE = os.environ.get("KERNEL_DTYPE", "bf16")  # "bf16" | "f32"
TRACE = os.environ.get("KERNEL_TRACE", "0") == "1"
# engines used for one-hot builds, cycled per subtile
OH_ENGINES = os.environ.get("KERNEL_OH_ENGINES", "vector,gpsimd,vector").split(",")


def _npdt():
    return ml_dtypes.bfloat16 if COMPUTE == "bf16" else np.float32


# ---------------------------------------------------------------- host plan
def _plan(seg_ids, n_segments):
    """Partition a scatter into per-core block schedules with a shared profile.

    Returns dict with:
      prof      [NB]              subtile count per block position (same all cores)
      slot      [NCORES, T*P]     source-row id per slot, -1 = padding
      offs      [NCORES, T*P]     local segment offset (0..127) per slot, -1 pad
      invc      [NCORES, NB, P]   1/max(count,1) per segment
      cnts      [NCORES, NB, P]   count per segment (0 for dummy blocks)
      outmap    [NCORES, NB]      real block id or -1
      nblk      total real blocks
    """
    n = seg_ids.shape[0]
    order = np.argsort(seg_ids, kind="stable")
    counts = np.bincount(seg_ids, minlength=n_segments).astype(np.int64)
    nblk = -(-n_segments // P)
    counts_p = np.zeros(nblk * P, np.int64)
    counts_p[:n_segments] = counts
    blk_cnt = counts_p.reshape(nblk, P).sum(1)
    nsub = np.maximum(1, -(-blk_cnt // P)).astype(np.int64)
    seg_start = np.zeros(nblk * P + 1, np.int64)
    np.cumsum(counts_p, out=seg_start[1:])

    per_core = [[] for _ in range(NCORES)]
    for nv in sorted(set(nsub.tolist()), reverse=True):
        bids = np.where(nsub == nv)[0].tolist()
        while len(bids) % NCORES:
            bids.append(-1)
        for i, bid in enumerate(bids):
            per_core[i % NCORES].append((int(bid), int(nv)))

    prof = [nv for (_, nv) in per_core[0]]
    NB = len(prof)
    T = sum(prof)
    slot = np.full((NCORES, T * P), -1, np.int64)
    offs = np.full((NCORES, T * P), -1.0, np.float32)
    invc = np.ones((NCORES, NB, P), np.float32)
    cnts = np.zeros((NCORES, NB, P), np.float32)
    outmap = np.full((NCORES, NB), -1, np.int64)
    for c in range(NCORES):
        t0 = 0
        for j, (bid, nv) in enumerate(per_core[c]):
            assert prof[j] == nv
            if bid >= 0:
                a = int(seg_start[bid * P])
                cnt = int(blk_cnt[bid])
                rows = order[a : a + cnt]
                slot[c, t0 * P : t0 * P + cnt] = rows
                offs[c, t0 * P : t0 * P + cnt] = (
                    seg_ids[rows] - bid * P
                ).astype(np.float32)
                invc[c, j] = 1.0 / np.maximum(counts_p[bid * P : (bid + 1) * P], 1)
                cnts[c, j] = counts_p[bid * P : (bid + 1) * P]
                outmap[c, j] = bid
            t0 += nv
    return dict(
        prof=prof, slot=slot, offs=offs, invc=invc, cnts=cnts, outmap=outmap,
        nblk=nblk, NB=NB, T=T,
    )


def _gather_block_layout(V, slot, dt):
    """V [N, H] -> [P, T*H] where out[e, t*H + f] = V[slot[t*P + e], f] (0 pad)."""
    TP = slot.shape[0]
    T = TP // P
    out = np.zeros((TP, V.shape[1]), dtype=dt)
    m = slot >= 0
    out[m] = V[slot[m]].astype(dt)
    return np.ascontiguousarray(
        out.reshape(T, P, V.shape[1]).transpose(1, 0, 2).reshape(P, T * V.shape[1])
    )


def _offs_layout(offs, dt):
    """[T*P] -> [P, T] with out[p, t] = offs[t*P + p]."""
    T = offs.shape[0] // P
    return np.ascontiguousarray(offs.reshape(T, P).T.astype(dt))


# ---------------------------------------------------------------- device program
def _build_program(prof_e, NB_e, prof_n, NB_n):
    from contextlib import ExitStack

    import concourse.tile as tile
    from concourse import bacc, bass, mybir

    DT = mybir.dt.bfloat16 if COMPUTE == "bf16" else mybir.dt.float32
    F32 = mybir.dt.float32
    T_e = sum(prof_e)
    T_n = sum(prof_n)

    nc = bacc.Bacc(
        "TRN2", target_bir_lowering=False, debug=False, num_devices=NCORES
    )

    def din(name, shape, dt=DT):
        return nc.declare_dram_parameter(name, list(shape), dt, isOutput=False).ap()

    vals_e = din("vals_e", [P, T_e * H])
    offs_e = din("offs_e", [P, T_e])
    invc_e = din("invc_e", [P, NB_e], F32)
    xaug = din("xaug", [P + 2, T_n * P])
    offs_n = din("offs_n", [P, T_n])
    invc_n = din("invc_n", [P, NB_n], F32)
    cnt_n = din("cnt_n", [1, NB_n * P])
    w1a = din("w1a", [P, P])
    w1b = din("w1b", [2, P])
    w2 = din("w2", [P, P])
    b2r = din("b2r", [1, P])
    iota_d = din("iota_d", [P, P])
    ident_d = din("ident_d", [P, P])

    ce_out = nc.declare_dram_parameter(
        "ce_out", [P, NB_e * P], F32, isOutput=True
    ).ap()
    cn_out = nc.declare_dram_parameter(
        "cn_out", [P, NB_n * P], F32, isOutput=True
    ).ap()

    eq = mybir.AluOpType.is_equal
    Relu = mybir.ActivationFunctionType.Relu

    def oh_engine(t):
        name = OH_ENGINES[t % len(OH_ENGINES)]
        return getattr(nc, name)

    with tile.TileContext(nc) as tc, ExitStack() as ctx:
        const = ctx.enter_context(tc.tile_pool(name="const", bufs=1))

        iota_t = const.tile([P, P], DT)
        ident_t = const.tile([P, P], DT)
        w1a_t = const.tile([P, P], DT)
        w1b_t = const.tile([2, P], DT)
        w2_t = const.tile([P, P], DT)
        b2_t = const.tile([1, P], DT)
        offs_e_t = const.tile([P, T_e], DT)
        invc_e_t = const.tile([P, NB_e], F32)
        offs_n_t = const.tile([P, T_n], DT)
        invc_n_t = const.tile([P, NB_n], F32)
        cnt_n_t = const.tile([1, NB_n * P], DT)
        for dst, src in [
            (iota_t, iota_d), (ident_t, ident_d), (w1a_t, w1a), (w1b_t, w1b),
            (w2_t, w2), (b2_t, b2r), (offs_e_t, offs_e), (invc_e_t, invc_e),
            (offs_n_t, offs_n), (invc_n_t, invc_n), (cnt_n_t, cnt_n),
        ]:
            nc.sync.dma_start(out=dst[:], in_=src[:])

        # ---------------- edge phase -------------------------------------
        with tc.tile_pool(name="ev", bufs=3) as vpool, \
             tc.tile_pool(name="eo", bufs=8) as ohpool, \
             tc.tile_pool(name="eout", bufs=3) as outpool, \
             tc.tile_pool(name="eps", bufs=4, space="PSUM") as epsum:
            t = 0
            outt = None
            for b in range(NB_e):
                n = prof_e[b]
                vals = vpool.tile([P, n * H], DT, tag="ev")
                nc.sync.dma_start(out=vals[:], in_=vals_e[:, t * H : (t + n) * H])
                ps = epsum.tile([P, P], F32, tag="eps")
                for s in range(n):
                    oh = ohpool.tile([P, P], DT, tag="eo")
                    oh_engine(t).tensor_tensor(
                        oh[:], iota_t[:],
                        offs_e_t[:, t : t + 1].to_broadcast([P, P]), eq
                    )
                    nc.tensor.matmul(
                        ps[:], lhsT=oh[:], rhs=vals[:, s * H : (s + 1) * H],
                        start=(s == 0), stop=(s == n - 1),
                    )
                    t += 1
                ob = b % OUT_BATCH
                if ob == 0:
                    outt = outpool.tile([P, OUT_BATCH * P], F32, tag="eout")
                nc.scalar.mul(
                    outt[:, ob * P : (ob + 1) * P], ps[:], invc_e_t[:, b : b + 1]
                )
                if ob == OUT_BATCH - 1 or b == NB_e - 1:
                    c0 = (b - ob) * P
                    nc.sync.dma_start(
                        out=ce_out[:, c0 : (b + 1) * P], in_=outt[:, : (ob + 1) * P]
                    )

        # ---------------- node phase -------------------------------------
        with tc.tile_pool(name="nx", bufs=3) as xpool, \
             tc.tile_pool(name="nd", bufs=3) as dpool, \
             tc.tile_pool(name="nh", bufs=4) as hpool, \
             tc.tile_pool(name="nf", bufs=4) as fpool, \
             tc.tile_pool(name="nv", bufs=4) as nvpool, \
             tc.tile_pool(name="no", bufs=8) as nohpool, \
             tc.tile_pool(name="nout", bufs=3) as noutpool, \
             tc.tile_pool(name="mps", bufs=4, space="PSUM") as mpsum, \
             tc.tile_pool(name="tps", bufs=2, space="PSUM") as tpsum, \
             tc.tile_pool(name="cps", bufs=2, space="PSUM") as cpsum:
            t = 0
            outt = None
            for b in range(NB_n):
                n = prof_n[b]
                xs = xpool.tile([P, n * P], DT, tag="nx")
                nc.sync.dma_start(out=xs[:], in_=xaug[0:P, t * P : (t + n) * P])
                da = dpool.tile([2, n * P], DT, tag="nd")
                nc.sync.dma_start(
                    out=da[:], in_=xaug[P : P + 2, t * P : (t + n) * P]
                )
                cps = cpsum.tile([P, P], F32, tag="cps")
                # mean(x + mlp + b2) = (sum(x + mlp) + count*b2) * invc
                # out[seg, feat] += cnt[seg] * b2[feat]
                nc.tensor.matmul(
                    cps[:], lhsT=cnt_n_t[:, b * P : (b + 1) * P], rhs=b2_t[:],
                    start=True, stop=False,
                )
                for s in range(n):
                    sl = slice(s * P, (s + 1) * P)
                    ps1 = mpsum.tile([P, P], F32, tag="mps")
                    nc.tensor.matmul(
                        ps1[:], lhsT=w1a_t[:], rhs=xs[:, sl], start=True, stop=False
                    )
                    nc.tensor.matmul(
                        ps1[:], lhsT=w1b_t[:], rhs=da[:, sl], start=False, stop=True
                    )
                    h = hpool.tile([P, P], DT, tag="nh")
                    nc.scalar.activation(h[:], ps1[:], Relu)
                    ps2 = mpsum.tile([P, P], F32, tag="mps")
                    nc.tensor.matmul(
                        ps2[:], lhsT=w2_t[:], rhs=h[:], start=True, stop=True
                    )
                    ft = fpool.tile([P, P], DT, tag="nf")
                    nc.vector.tensor_tensor(
                        ft[:], ps2[:], xs[:, sl], mybir.AluOpType.add
                    )
                    tp = tpsum.tile([P, P], DT, tag="tps")
                    nc.tensor.transpose(tp[:], ft[:], ident_t[:])
                    vm = nvpool.tile([P, P], DT, tag="nv")
                    if t % 2 == 0:
                        nc.scalar.copy(vm[:], tp[:])
                    else:
                        nc.vector.tensor_copy(vm[:], tp[:])
                    oh = nohpool.tile([P, P], DT, tag="no")
                    oh_engine(t).tensor_tensor(
                        oh[:], iota_t[:],
                        offs_n_t[:, t : t + 1].to_broadcast([P, P]), eq
                    )
                    nc.tensor.matmul(
                        cps[:], lhsT=oh[:], rhs=vm[:],
                        start=False, stop=(s == n - 1),
                    )
                    t += 1
                ob = b % OUT_BATCH
                if ob == 0:
                    outt = noutpool.tile([P, OUT_BATCH * P], F32, tag="nout")
                nc.scalar.mul(
                    outt[:, ob * P : (ob + 1) * P], cps[:], invc_n_t[:, b : b + 1]
                )
                if ob == OUT_BATCH - 1 or b == NB_n - 1:
                    c0 = (b - ob) * P
                    nc.sync.dma_start(
                        out=cn_out[:, c0 : (b + 1) * P], in_=outt[:, : (ob + 1) * P]
                    )
    if TRACE:
        try:
            mx = 0
            per_proc = {}
            for blk in nc.m.functions[0].blocks:
                for ins in blk.instructions:
                    tk = getattr(ins, "bass_scheduled_tick", None)
                    pr = getattr(ins, "bass_scheduled_proc", None)
                    if tk:
                        mx = max(mx, tk)
                        if pr is not None:
                            per_proc[str(pr)] = max(per_proc.get(str(pr), 0), tk)
            print(f"cost-model est span: {mx} ns")
            print("per-proc last tick:",
                  dict(sorted(per_proc.items(), key=lambda kv: -kv[1])[:8]))
        except Exception as e:
            print("tick extract failed:", e)
    nc.compile()
    return nc


# ---------------------------------------------------------------- executor
def _execute(nc, in_maps):
    """Run the SPMD program on 8 cores via PJRT (axon).  Mirrors
    bass2jax.run_bass_via_pjrt, but pre-places inputs on device and, when
    TRACE is set, wall-clock-times warm re-executions (no NTFF hook in this
    container)."""
    import time as _time

    import jax
    from jax.experimental.shard_map import shard_map
    from jax.sharding import Mesh, NamedSharding, PartitionSpec

    from concourse import bass2jax, mybir

    bass2jax.install_neuronx_cc_hook()

    part_name = nc.partition_id_tensor.name if nc.partition_id_tensor else None
    in_names, out_names, out_avals, zero_outs = [], [], [], []
    for alloc in nc.m.functions[0].allocations:
        if not isinstance(alloc, mybir.MemoryLocationSet):
            continue
        name = alloc.memorylocations[0].name
        if alloc.kind == "ExternalInput":
            if name != part_name:
                in_names.append(name)
        elif alloc.kind == "ExternalOutput":
            out_names.append(name)
            shape = tuple(alloc.tensor_shape)
            dtype = mybir.dt.np(alloc.dtype)
            out_avals.append(jax.core.ShapedArray(shape, dtype))
            zero_outs.append(np.zeros(shape, dtype))
    n_params = len(in_names)
    all_names = in_names + out_names
    if part_name is not None:
        all_names = all_names + [part_name]

    def _body(*args):
        operands = list(args)
        if part_name is not None:
            operands.append(bass2jax.partition_id_tensor())
        outs = bass2jax._bass_exec_p.bind(
            *operands,
            out_avals=tuple(out_avals),
            in_names=tuple(all_names),
            out_names=tuple(out_names),
            lowering_input_output_aliases=(),
            sim_require_finite=True,
            sim_require_nnan=True,
            nc=nc,
        )
        return tuple(outs)

    devices = jax.devices()[:NCORES]
    mesh = Mesh(np.asarray(devices), ("core",))
    spec = NamedSharding(mesh, PartitionSpec("core"))
    nin = n_params + len(out_names)
    sharded = jax.jit(
        shard_map(
            _body, mesh=mesh, in_specs=(PartitionSpec("core"),) * nin,
            out_specs=(PartitionSpec("core"),) * len(out_names), check_rep=False,
        ),
        keep_unused=True,
    )
    concat_in = [
        jax.device_put(
            np.concatenate([np.asarray(in_maps[c][nm]) for c in range(NCORES)], 0),
            spec,
        )
        for nm in in_names
    ]
    concat_zero = [
        jax.device_put(np.zeros((NCORES * z.shape[0], *z.shape[1:]), z.dtype), spec)
        for z in zero_outs
    ]
    out_arrs = jax.block_until_ready(sharded(*concat_in, *concat_zero))
    if TRACE:
        times = []
        for _ in range(5):
            t0 = _time.perf_counter()
            jax.block_until_ready(sharded(*concat_in, *concat_zero))
            times.append(_time.perf_counter() - t0)
        print(f"HW exec time: {int(min(times) * 1e9)} ns")
        print("exec wall times (ms):", [round(t * 1e3, 3) for t in times])
    return [
        {
            nm: np.asarray(out_arrs[i]).reshape(NCORES, *out_avals[i].shape)[c]
            for i, nm in enumerate(out_names)
        }
        for c in range(NCORES)
    ]


# ---------------------------------------------------------------- runner
def _run(x, coarse_edge_index, edge_attr, fine2coarse_index, fine2coarse_edges,
         distances, W1, b1, W2, b2, n_coarse_nodes, n_coarse_edges):
    dt = _npdt()
    plan_e = _plan(fine2coarse_edges, n_coarse_edges)
    plan_n = _plan(fine2coarse_index, n_coarse_nodes)

    iota = np.broadcast_to(np.arange(P, dtype=np.float32), (P, P)).astype(dt)
    ident = np.eye(P, dtype=np.float32).astype(dt)
    w1a = W1[:P].astype(dt)
    w1b = np.stack([W1[P], b1]).astype(dt)  # [2, P]: distance row + bias row
    w2c = W2.astype(dt)
    b2r = b2[None, :].astype(dt)

    in_maps = []
    for c in range(NCORES):
        # node phase wants feature-major: xs[f, slot] = x[slot_row, f]
        slot_n = plan_n["slot"][c]
        m = slot_n >= 0
        g = np.zeros((slot_n.shape[0], H), np.float32)
        g[m] = x[slot_n[m]]
        xs = np.ascontiguousarray(g.T.astype(dt))  # [H, T_n*P]
        drow = np.zeros(slot_n.shape[0], np.float32)
        drow[m] = distances[slot_n[m], 0]
        ones = m.astype(np.float32)
        daux = np.stack([drow, ones]).astype(dt)  # [2, T_n*P] in slot order
        xaug = np.concatenate([xs, daux], axis=0)
        in_maps.append({
            "vals_e": _gather_block_layout(edge_attr, plan_e["slot"][c], dt),
            "offs_e": _offs_layout(plan_e["offs"][c], dt),
            "invc_e": np.ascontiguousarray(plan_e["invc"][c].T),
            "xaug": np.ascontiguousarray(xaug),
            "offs_n": _offs_layout(plan_n["offs"][c], dt),
            "invc_n": np.ascontiguousarray(plan_n["invc"][c].T),
            "cnt_n": plan_n["cnts"][c].reshape(1, -1).astype(dt),
            "w1a": w1a, "w1b": w1b, "w2": w2c, "b2r": b2r,
            "iota_d": iota, "ident_d": ident,
        })

    nc = _build_program(plan_e["prof"], plan_e["NB"], plan_n["prof"], plan_n["NB"])
    results = _execute(nc, in_maps)

    def assemble(key, plan, n_segments):
        nblk = plan["nblk"]
        NB = plan["NB"]
        full = np.zeros((nblk * P, H), np.float32)
        fullb = full.reshape(nblk, P, H)
        for c in range(NCORES):
            r = results[c][key]  # [P, NB*P]
            r = r.reshape(P, NB, P).transpose(1, 0, 2)  # [NB, seg_in_block, feat]
            msk = plan["outmap"][c] >= 0
            fullb[plan["outmap"][c][msk]] = r[msk]
        return full[:n_segments]

    ce = assemble("ce_out", plan_e, n_coarse_edges)
    cn = assemble("cn_out", plan_n, n_coarse_nodes)
    return cn, ce


def kernel(**inputs):
    x = np.asarray(inputs["x"], np.float32)
    cei = np.asarray(inputs["coarse_edge_index"])
    ea = np.asarray(inputs["edge_attr"], np.float32)
    f2ci = np.asarray(inputs["fine2coarse_index"]).astype(np.int64)
    f2ce = np.asarray(inputs["fine2coarse_edges"]).astype(np.int64)
    dist = np.asarray(inputs["distances"], np.float32)
    W1 = np.asarray(inputs["W1"], np.float32)
    b1 = np.asarray(inputs["b1"], np.float32)
    W2 = np.asarray(inputs["W2"], np.float32)
    b2 = np.asarray(inputs["b2"], np.float32)
    cn, ce = _run(x, cei, ea, f2ci, f2ce, dist, W1, b1, W2, b2,
                  N_COARSE, E_COARSE)
    return cn, ce, cei
